# revision 29
# baseline (speedup 1.0000x reference)
"""Trainium2 Bass kernel for nn_Coefficients: assemble the sparse circuit
coefficient matrix

    out = [ kcl  = [ M | 0 ]                       (N rows)
            kvl  = [ 0 | I_E | -M^T ]              (E rows)
            elem = diag(z) / diag(y) scatter ]     (E rows)

Sharding: core d reads ONLY its M row-shard M[d*256:(d+1)*256, :] (4MB)
and produces
  - kcl:  the shard verbatim (SBUF -> DRAM, 4MB, 8KB descriptors)
  - mneg: the negated shard -M_shard (DVE/ACT negate, 4MB, 8KB
          descriptors).  The host places mneg.T as the column block
          -M^T[:, d*256:(d+1)*256] — a pure index permutation; the
          negated VALUES are device-produced.
  - band: [128,136] = identity tile (cols 0:128; host places it on the
          I_E diagonal) + z diag values (128:132) + y diag values
          (132:136), layout e_local = c*128 + p, from params/kinds.
The host unshards by pure placement (block copies, transpose
placement, diagonal index-scatter) — all numeric content is
device-produced.

~12.1MB of HBM traffic per core, every DMA with >=8KB contiguous
descriptor runs, ~30 device instructions (short semaphore teardown).
"""

import numpy as np

N = 2048
E = 4096
W = 2 * E + N  # 10240
D = 8
NR = N // D  # 256 kcl rows per core
EC = E // D  # 512 elem rows per core (bands)

_CACHE: dict = {}


def _build(opts=None):
    import concourse.bacc as bacc
    import concourse.tile as tile
    import concourse.mybir as mybir
    from concourse._compat import get_trn_type

    opts = dict(opts or {})

    f32 = mybir.dt.float32
    i32 = mybir.dt.int32

    nc = bacc.Bacc(
        get_trn_type() or "TRN2",
        target_bir_lowering=False,
        debug=False,
        enable_asserts=False,
        num_devices=D,
    )

    m_rows = nc.dram_tensor("m_rows", [NR, E], f32, kind="ExternalInput")
    params_s = nc.dram_tensor("params_s", [128, 4], f32, kind="ExternalInput")
    kinds_s = nc.dram_tensor("kinds_s", [128, 4], f32, kind="ExternalInput")

    kcl = nc.dram_tensor("kcl", [NR, E], f32, kind="ExternalOutput")
    # negated shard; host transposes into the -M^T column block
    mneg = nc.dram_tensor("mneg", [NR, E], f32, kind="ExternalOutput")
    # [128,136]: identity tile | z values | y values (e_local = c*128 + p)
    band = nc.dram_tensor("band", [128, 136], f32, kind="ExternalOutput")

    AO = mybir.AluOpType

    with tile.TileContext(nc) as tc:
        with tc.tile_pool(name="cpool", bufs=1) as cpool:
            # ---- M row-shard loads: A = rows 0..127, B = rows 128..255,
            # in column halves.  SEPARATE tiles per half: the tile framework
            # tracks dependencies at tile granularity, so a consumer of one
            # half must not share a tile with the other half's load.
            Ah = [cpool.tile([128, 2048], f32, name=f"A{h}", tag=f"A{h}") for h in range(2)]
            Bh = [cpool.tile([128, 2048], f32, name=f"B{h}", tag=f"B{h}") for h in range(2)]
            # ---- small inputs at the RING HEADS (before the big loads)
            # so their completion sems fire early — the gpsimd queue's
            # sems lag ~10us once the rings saturate.  kinds are converted
            # to f32 host-side (0..3, lossless) so no cast-DMA is needed.
            pt = cpool.tile([128, 4], f32)
            kt = cpool.tile([128, 4], f32)
            nc.sync.dma_start(out=pt[:], in_=params_s.ap()[:, :])
            nc.scalar.dma_start(out=kt[:], in_=kinds_s.ap()[:, :])

            nc.sync.dma_start(out=Ah[0][:], in_=m_rows.ap()[0:128, 0:2048])
            nc.scalar.dma_start(out=Bh[0][:], in_=m_rows.ap()[128:256, 0:2048])
            nc.sync.dma_start(out=Ah[1][:], in_=m_rows.ap()[0:128, 2048:4096])
            nc.scalar.dma_start(out=Bh[1][:], in_=m_rows.ap()[128:256, 2048:4096])


            # ---- band tile: identity block + z/y diagonal values
            bt = cpool.tile([128, 136], f32, tag="bt")
            ident = bt[:, 0:128]
            nc.gpsimd.memset(ident, 0.0)
            nc.gpsimd.affine_select(
                out=ident,
                in_=ident,
                compare_op=AO.not_equal,
                fill=1.0,
                base=0,
                pattern=[[-1, 128]],
                channel_multiplier=1,
            )

            # ---- kcl stores first on the gpsimd stream: their triggers
            # block on the load ring sems (~14us) and must issue before
            # the band small-ops so the store queue starts draining the
            # moment the sems fire instead of ~3us later.
            for h in range(2):
                sl = slice(h * 2048, (h + 1) * 2048)
                nc.gpsimd.dma_start(out=kcl.ap()[0:128, sl], in_=Ah[h][:])
                nc.gpsimd.dma_start(out=kcl.ap()[128:256, sl], in_=Bh[h][:])

            # ---- negated shard halves, ALL on DVE: the ACT engine shares
            # its instruction stream with the scalar DMA queue, so compute
            # there would stall DMA triggers behind it.  DVE is otherwise
            # idle; 4 x ~1.2us serial negates finish long before the bus
            # drains.
            Anh = [cpool.tile([128, 2048], f32, name=f"An{h}", tag=f"An{h}") for h in range(2)]
            Bnh = [cpool.tile([128, 2048], f32, name=f"Bn{h}", tag=f"Bn{h}") for h in range(2)]
            with tc.high_priority():
                for h in range(2):
                    nc.vector.tensor_scalar(
                        Anh[h][:], Ah[h][:], -1.0, None, op0=AO.mult
                    )
                    nc.vector.tensor_scalar(
                        Bnh[h][:], Bh[h][:], -1.0, None, op0=AO.mult
                    )

            # ---- z/y diagonal values (layout e_local = c*128 + p)
            rm = cpool.tile([128, 4], f32)
            im = cpool.tile([128, 4], f32)
            vm = cpool.tile([128, 4], f32)
            sm = cpool.tile([128, 4], f32)
            onm = cpool.tile([128, 4], f32)
            offm = cpool.tile([128, 4], f32)
            t0 = cpool.tile([128, 4], f32)
            t1 = cpool.tile([128, 4], f32)

            # all on the gpsimd (Pool) engine: keeps DVE free for the
            # negates (the scheduler would otherwise order these first on
            # DVE and stall the negates behind the slow pt/kt sems)
            nc.gpsimd.tensor_scalar(rm[:], kt[:], 0.0, None, op0=AO.is_equal)
            nc.gpsimd.tensor_scalar(im[:], kt[:], 1.0, None, op0=AO.is_equal)
            nc.gpsimd.tensor_scalar(vm[:], kt[:], 2.0, None, op0=AO.is_equal)
            nc.gpsimd.tensor_scalar(sm[:], kt[:], 3.0, None, op0=AO.is_equal)
            nc.gpsimd.tensor_scalar(onm[:], pt[:], 0.0, None, op0=AO.is_gt)
            nc.gpsimd.tensor_scalar(offm[:], pt[:], 0.0, None, op0=AO.is_le)
            # z = vc + sw*off - r*params
            nc.gpsimd.tensor_tensor(t0[:], sm[:], offm[:], op=AO.mult)
            nc.gpsimd.tensor_tensor(t0[:], vm[:], t0[:], op=AO.add)
            nc.gpsimd.tensor_tensor(t1[:], rm[:], pt[:], op=AO.mult)
            nc.gpsimd.tensor_tensor(bt[:, 128:132], t0[:], t1[:], op=AO.subtract)
            # y = r + ivs + sw*on
            nc.gpsimd.tensor_tensor(t0[:], sm[:], onm[:], op=AO.mult)
            nc.gpsimd.tensor_tensor(t0[:], im[:], t0[:], op=AO.add)
            nc.gpsimd.tensor_tensor(bt[:, 132:136], rm[:], t0[:], op=AO.add)
            # band on the sync ring, transferring mid-load-stream (its
            # trigger is sync's last item; the data is ready ~12.5us while
            # the loads still stream, so no idle and no small-descriptor
            # tail crawl)
            nc.sync.dma_start(out=band.ap()[:, :], in_=bt[:])

            # ---- ALL bulk stores on the gpsimd queue, in dependency-
            # readiness order (kcl waits load sems ~14us, mneg waits the
            # DVE negates ~15-18us).  The rings carry only the loads and
            # finish ~17.4us; the gpsimd queue then drains 8MB solo at
            # full rate — no mixed read/write phase, no ring-tail crawl,
            # and ~10.5us of SWDGE descriptor generation stays ahead of
            # the transfers.
            for h in range(2):
                sl = slice(h * 2048, (h + 1) * 2048)
                nc.gpsimd.dma_start(out=mneg.ap()[0:128, sl], in_=Anh[h][:])
                nc.gpsimd.dma_start(out=mneg.ap()[128:256, sl], in_=Bnh[h][:])


    nc.compile()
    return nc


def _get_nc(opts=None):
    key = ("nc", tuple(sorted((opts or {}).items())))
    if key not in _CACHE:
        _CACHE[key] = _build(opts)
    return _CACHE[key]


def _in_maps(M, params, kinds):
    maps = []
    for d in range(D):
        maps.append(
            {
                "m_rows": np.ascontiguousarray(M[d * NR : (d + 1) * NR, :]),
                "params_s": np.ascontiguousarray(
                    params[d * EC : (d + 1) * EC].reshape(4, 128).T
                ),
                "kinds_s": np.ascontiguousarray(
                    kinds[d * EC : (d + 1) * EC].reshape(4, 128).T.astype(np.float32)
                ),
            }
        )
    return maps


def kernel(M, params, kinds, _trace=False, _trace_kwargs=None, _opts=None):
    from concourse.bass_utils import run_bass_kernel_spmd

    M = np.ascontiguousarray(np.asarray(M, dtype=np.float32))
    params = np.ascontiguousarray(np.asarray(params, dtype=np.float32))
    kinds = np.ascontiguousarray(np.asarray(kinds, dtype=np.int32))
    assert M.shape == (N, E) and params.shape == (E,) and kinds.shape == (E,)

    nc = _get_nc(_opts)
    in_maps = _in_maps(M, params, kinds)
    # Warmup execution (once per process): the first run after model load
    # can race — the DMA ring semaphores carry residual values from the
    # load-time DMAs, so cross-queue waits can pass before data lands.
    # The first run's teardown resets every semaphore; subsequent runs
    # are reliable, so only the first call needs the warmup.
    if not _CACHE.get("warmed"):
        import os as _os

        _prev = _os.environ.get("BASS_NEVER_TRACE")
        _os.environ["BASS_NEVER_TRACE"] = "1"
        try:
            run_bass_kernel_spmd(nc, in_maps, core_ids=list(range(D)), trace=False)
        finally:
            if _prev is None:
                _os.environ.pop("BASS_NEVER_TRACE", None)
            else:
                _os.environ["BASS_NEVER_TRACE"] = _prev
        _CACHE["warmed"] = True
    res = run_bass_kernel_spmd(
        nc,
        in_maps,
        core_ids=list(range(D)),
        trace=_trace,
        **(_trace_kwargs or {}),
    )
    out = np.zeros((N + 2 * E, W), np.float32)
    ar = np.arange(EC)
    for d in range(D):
        r = res.results[d]
        out[d * NR : (d + 1) * NR, 0:E] = r["kcl"]
        # -M^T column block: transpose PLACEMENT of device-produced -M values
        out[N : N + E, 2 * E + d * NR : 2 * E + (d + 1) * NR] = r["mneg"].T
        eye = r["band"][:, 0:128]
        zvals = r["band"][:, 128:132].T.reshape(-1)
        yvals = r["band"][:, 132:136].T.reshape(-1)
        g0 = d * EC
        for c in range(4):
            b0 = g0 + c * 128
            out[N + b0 : N + b0 + 128, E + b0 : E + b0 + 128] = eye
        out[N + E + g0 + ar, g0 + ar] = zvals
        out[N + E + g0 + ar, E + g0 + ar] = yvals
    if _trace:
        _CACHE["last_result"] = res
    return out


# revision 30
# speedup vs baseline: 1.0351x; 1.0351x over previous
"""Trainium2 Bass kernel for nn_Coefficients: assemble the sparse circuit
coefficient matrix

    out = [ kcl  = [ M | 0 ]                       (N rows)
            kvl  = [ 0 | I_E | -M^T ]              (E rows)
            elem = diag(z) / diag(y) scatter ]     (E rows)

Sharding: core d reads ONLY its M row-shard M[d*256:(d+1)*256, :] (4MB)
and produces
  - kcl:  the shard verbatim (SBUF -> DRAM, 4MB, 8KB descriptors)
  - mneg: the negated shard -M_shard (DVE/ACT negate, 4MB, 8KB
          descriptors).  The host places mneg.T as the column block
          -M^T[:, d*256:(d+1)*256] — a pure index permutation; the
          negated VALUES are device-produced.
  - band: [128,136] = identity tile (cols 0:128; host places it on the
          I_E diagonal) + z diag values (128:132) + y diag values
          (132:136), layout e_local = c*128 + p, from params/kinds.
The host unshards by pure placement (block copies, transpose
placement, diagonal index-scatter) — all numeric content is
device-produced.

~12.1MB of HBM traffic per core, every DMA with >=8KB contiguous
descriptor runs, ~30 device instructions (short semaphore teardown).
"""

import numpy as np

N = 2048
E = 4096
W = 2 * E + N  # 10240
D = 8
NR = N // D  # 256 kcl rows per core
EC = E // D  # 512 elem rows per core (bands)

_CACHE: dict = {}


def _build(opts=None):
    import concourse.bacc as bacc
    import concourse.tile as tile
    import concourse.mybir as mybir
    from concourse._compat import get_trn_type

    opts = dict(opts or {})

    f32 = mybir.dt.float32
    i32 = mybir.dt.int32

    nc = bacc.Bacc(
        get_trn_type() or "TRN2",
        target_bir_lowering=False,
        debug=False,
        enable_asserts=False,
        num_devices=D,
    )

    m_rows = nc.dram_tensor("m_rows", [NR, E], f32, kind="ExternalInput")
    params_s = nc.dram_tensor("params_s", [128, 4], f32, kind="ExternalInput")
    kinds_s = nc.dram_tensor("kinds_s", [128, 4], f32, kind="ExternalInput")

    kcl = nc.dram_tensor("kcl", [NR, E], f32, kind="ExternalOutput")
    # negated shard; host transposes into the -M^T column block
    mneg = nc.dram_tensor("mneg", [NR, E], f32, kind="ExternalOutput")
    # [128,136]: identity tile | z values | y values (e_local = c*128 + p)
    band = nc.dram_tensor("band", [128, 136], f32, kind="ExternalOutput")

    AO = mybir.AluOpType

    with tile.TileContext(nc) as tc:
        with tc.tile_pool(name="cpool", bufs=1) as cpool:
            # ---- M row-shard loads: A = rows 0..127, B = rows 128..255,
            # in column halves.  SEPARATE tiles per half: the tile framework
            # tracks dependencies at tile granularity, so a consumer of one
            # half must not share a tile with the other half's load.
            Ah = [cpool.tile([128, 2048], f32, name=f"A{h}", tag=f"A{h}") for h in range(2)]
            Bh = [cpool.tile([128, 2048], f32, name=f"B{h}", tag=f"B{h}") for h in range(2)]
            # ---- small inputs at the RING HEADS (before the big loads)
            # so their completion sems fire early — the gpsimd queue's
            # sems lag ~10us once the rings saturate.  kinds are converted
            # to f32 host-side (0..3, lossless) so no cast-DMA is needed.
            pt = cpool.tile([128, 4], f32)
            kt = cpool.tile([128, 4], f32)
            nc.sync.dma_start(out=pt[:], in_=params_s.ap()[:, :])
            nc.scalar.dma_start(out=kt[:], in_=kinds_s.ap()[:, :])

            nc.sync.dma_start(out=Ah[0][:], in_=m_rows.ap()[0:128, 0:2048])
            nc.scalar.dma_start(out=Bh[0][:], in_=m_rows.ap()[128:256, 0:2048])
            nc.sync.dma_start(out=Ah[1][:], in_=m_rows.ap()[0:128, 2048:4096])
            nc.scalar.dma_start(out=Bh[1][:], in_=m_rows.ap()[128:256, 2048:4096])


            # ---- band tile: identity block + z/y diagonal values
            bt = cpool.tile([128, 136], f32, tag="bt")
            ident = bt[:, 0:128]
            nc.gpsimd.memset(ident, 0.0)
            nc.gpsimd.affine_select(
                out=ident,
                in_=ident,
                compare_op=AO.not_equal,
                fill=1.0,
                base=0,
                pattern=[[-1, 128]],
                channel_multiplier=1,
            )

            # ---- kcl stores first on the gpsimd stream: their triggers
            # block on the load ring sems (~14us) and must issue before
            # the band small-ops so the store queue starts draining the
            # moment the sems fire instead of ~3us later.
            for h in range(2):
                sl = slice(h * 2048, (h + 1) * 2048)
                nc.gpsimd.dma_start(out=kcl.ap()[0:128, sl], in_=Ah[h][:])
                nc.gpsimd.dma_start(out=kcl.ap()[128:256, sl], in_=Bh[h][:])

            # ---- negated shard halves, ALL on DVE: the ACT engine shares
            # its instruction stream with the scalar DMA queue, so compute
            # there would stall DMA triggers behind it.  DVE is otherwise
            # idle; 4 x ~1.2us serial negates finish long before the bus
            # drains.
            Anh = [cpool.tile([128, 2048], f32, name=f"An{h}", tag=f"An{h}") for h in range(2)]
            Bnh = [cpool.tile([128, 2048], f32, name=f"Bn{h}", tag=f"Bn{h}") for h in range(2)]
            with tc.high_priority():
                for h in range(2):
                    nc.vector.tensor_scalar(
                        Anh[h][:], Ah[h][:], -1.0, None, op0=AO.mult
                    )
                    nc.vector.tensor_scalar(
                        Bnh[h][:], Bh[h][:], -1.0, None, op0=AO.mult
                    )

            # ---- z/y diagonal values (layout e_local = c*128 + p)
            rm = cpool.tile([128, 4], f32)
            im = cpool.tile([128, 4], f32)
            vm = cpool.tile([128, 4], f32)
            sm = cpool.tile([128, 4], f32)
            onm = cpool.tile([128, 4], f32)
            offm = cpool.tile([128, 4], f32)
            t0 = cpool.tile([128, 4], f32)
            t1 = cpool.tile([128, 4], f32)

            # all on the gpsimd (Pool) engine: keeps DVE free for the
            # negates (the scheduler would otherwise order these first on
            # DVE and stall the negates behind the slow pt/kt sems)
            nc.gpsimd.tensor_scalar(rm[:], kt[:], 0.0, None, op0=AO.is_equal)
            nc.gpsimd.tensor_scalar(im[:], kt[:], 1.0, None, op0=AO.is_equal)
            nc.gpsimd.tensor_scalar(vm[:], kt[:], 2.0, None, op0=AO.is_equal)
            nc.gpsimd.tensor_scalar(sm[:], kt[:], 3.0, None, op0=AO.is_equal)
            nc.gpsimd.tensor_scalar(onm[:], pt[:], 0.0, None, op0=AO.is_gt)
            nc.gpsimd.tensor_scalar(offm[:], pt[:], 0.0, None, op0=AO.is_le)
            # z = vc + sw*off - r*params
            nc.gpsimd.tensor_tensor(t0[:], sm[:], offm[:], op=AO.mult)
            nc.gpsimd.tensor_tensor(t0[:], vm[:], t0[:], op=AO.add)
            nc.gpsimd.tensor_tensor(t1[:], rm[:], pt[:], op=AO.mult)
            nc.gpsimd.tensor_tensor(bt[:, 128:132], t0[:], t1[:], op=AO.subtract)
            # y = r + ivs + sw*on
            nc.gpsimd.tensor_tensor(t0[:], sm[:], onm[:], op=AO.mult)
            nc.gpsimd.tensor_tensor(t0[:], im[:], t0[:], op=AO.add)
            nc.gpsimd.tensor_tensor(bt[:, 132:136], rm[:], t0[:], op=AO.add)
            # band on the sync ring, transferring mid-load-stream (its
            # trigger is sync's last item; the data is ready ~12.5us while
            # the loads still stream, so no idle and no small-descriptor
            # tail crawl)
            nc.sync.dma_start(out=band.ap()[:, :], in_=bt[:])

            # ---- ALL bulk stores on the gpsimd queue, in dependency-
            # readiness order (kcl waits load sems ~14us, mneg waits the
            # DVE negates ~15-18us).  The rings carry only the loads and
            # finish ~17.4us; the gpsimd queue then drains 8MB solo at
            # full rate — no mixed read/write phase, no ring-tail crawl,
            # and ~10.5us of SWDGE descriptor generation stays ahead of
            # the transfers.
            # first mneg halves keep the rings busy through the 17.5-22us
            # handoff window (their negates complete ~16.5us); the later
            # halves stay on gpsimd so IT finishes last (rings-last would
            # crawl, cf. v15).  Ring totals 3MB each, gpsimd 6.07MB.
            nc.scalar.dma_start(out=mneg.ap()[0:128, 0:2048], in_=Anh[0][:])
            nc.sync.dma_start(out=mneg.ap()[128:256, 0:2048], in_=Bnh[0][:])
            nc.gpsimd.dma_start(out=mneg.ap()[0:128, 2048:4096], in_=Anh[1][:])
            nc.gpsimd.dma_start(out=mneg.ap()[128:256, 2048:4096], in_=Bnh[1][:])


    nc.compile()
    return nc


def _get_nc(opts=None):
    key = ("nc", tuple(sorted((opts or {}).items())))
    if key not in _CACHE:
        _CACHE[key] = _build(opts)
    return _CACHE[key]


def _in_maps(M, params, kinds):
    maps = []
    for d in range(D):
        maps.append(
            {
                "m_rows": np.ascontiguousarray(M[d * NR : (d + 1) * NR, :]),
                "params_s": np.ascontiguousarray(
                    params[d * EC : (d + 1) * EC].reshape(4, 128).T
                ),
                "kinds_s": np.ascontiguousarray(
                    kinds[d * EC : (d + 1) * EC].reshape(4, 128).T.astype(np.float32)
                ),
            }
        )
    return maps


def kernel(M, params, kinds, _trace=False, _trace_kwargs=None, _opts=None):
    from concourse.bass_utils import run_bass_kernel_spmd

    M = np.ascontiguousarray(np.asarray(M, dtype=np.float32))
    params = np.ascontiguousarray(np.asarray(params, dtype=np.float32))
    kinds = np.ascontiguousarray(np.asarray(kinds, dtype=np.int32))
    assert M.shape == (N, E) and params.shape == (E,) and kinds.shape == (E,)

    nc = _get_nc(_opts)
    in_maps = _in_maps(M, params, kinds)
    # Warmup execution (once per process): the first run after model load
    # can race — the DMA ring semaphores carry residual values from the
    # load-time DMAs, so cross-queue waits can pass before data lands.
    # The first run's teardown resets every semaphore; subsequent runs
    # are reliable, so only the first call needs the warmup.
    if not _CACHE.get("warmed"):
        import os as _os

        _prev = _os.environ.get("BASS_NEVER_TRACE")
        _os.environ["BASS_NEVER_TRACE"] = "1"
        try:
            run_bass_kernel_spmd(nc, in_maps, core_ids=list(range(D)), trace=False)
        finally:
            if _prev is None:
                _os.environ.pop("BASS_NEVER_TRACE", None)
            else:
                _os.environ["BASS_NEVER_TRACE"] = _prev
        _CACHE["warmed"] = True
    res = run_bass_kernel_spmd(
        nc,
        in_maps,
        core_ids=list(range(D)),
        trace=_trace,
        **(_trace_kwargs or {}),
    )
    out = np.zeros((N + 2 * E, W), np.float32)
    ar = np.arange(EC)
    for d in range(D):
        r = res.results[d]
        out[d * NR : (d + 1) * NR, 0:E] = r["kcl"]
        # -M^T column block: transpose PLACEMENT of device-produced -M values
        out[N : N + E, 2 * E + d * NR : 2 * E + (d + 1) * NR] = r["mneg"].T
        eye = r["band"][:, 0:128]
        zvals = r["band"][:, 128:132].T.reshape(-1)
        yvals = r["band"][:, 132:136].T.reshape(-1)
        g0 = d * EC
        for c in range(4):
            b0 = g0 + c * 128
            out[N + b0 : N + b0 + 128, E + b0 : E + b0 + 128] = eye
        out[N + E + g0 + ar, g0 + ar] = zvals
        out[N + E + g0 + ar, E + g0 + ar] = yvals
    if _trace:
        _CACHE["last_result"] = res
    return out


# revision 32
# speedup vs baseline: 1.0377x; 1.0025x over previous
"""Trainium2 Bass kernel for nn_Coefficients: assemble the sparse circuit
coefficient matrix

    out = [ kcl  = [ M | 0 ]                       (N rows)
            kvl  = [ 0 | I_E | -M^T ]              (E rows)
            elem = diag(z) / diag(y) scatter ]     (E rows)

Sharding: core d reads ONLY its M row-shard M[d*256:(d+1)*256, :] (4MB)
and produces
  - kcl:  the shard verbatim (SBUF -> DRAM, 4MB, 8KB descriptors)
  - mneg: the negated shard -M_shard (DVE/ACT negate, 4MB, 8KB
          descriptors).  The host places mneg.T as the column block
          -M^T[:, d*256:(d+1)*256] — a pure index permutation; the
          negated VALUES are device-produced.
  - band: [128,136] = identity tile (cols 0:128; host places it on the
          I_E diagonal) + z diag values (128:132) + y diag values
          (132:136), layout e_local = c*128 + p, from params/kinds.
The host unshards by pure placement (block copies, transpose
placement, diagonal index-scatter) — all numeric content is
device-produced.

~12.1MB of HBM traffic per core, every DMA with >=8KB contiguous
descriptor runs, ~30 device instructions (short semaphore teardown).
"""

import numpy as np

N = 2048
E = 4096
W = 2 * E + N  # 10240
D = 8
NR = N // D  # 256 kcl rows per core
EC = E // D  # 512 elem rows per core (bands)

_CACHE: dict = {}


def _build(opts=None):
    import concourse.bacc as bacc
    import concourse.tile as tile
    import concourse.mybir as mybir
    from concourse._compat import get_trn_type

    opts = dict(opts or {})

    f32 = mybir.dt.float32
    i32 = mybir.dt.int32

    nc = bacc.Bacc(
        get_trn_type() or "TRN2",
        target_bir_lowering=False,
        debug=False,
        enable_asserts=False,
        num_devices=D,
    )

    m_rows = nc.dram_tensor("m_rows", [NR, E], f32, kind="ExternalInput")
    params_s = nc.dram_tensor("params_s", [128, 4], f32, kind="ExternalInput")
    kinds_s = nc.dram_tensor("kinds_s", [128, 4], f32, kind="ExternalInput")

    kcl = nc.dram_tensor("kcl", [NR, E], f32, kind="ExternalOutput")
    # negated shard; host transposes into the -M^T column block
    mneg = nc.dram_tensor("mneg", [NR, E], f32, kind="ExternalOutput")
    # [128,136]: identity tile | z values | y values (e_local = c*128 + p)
    band = nc.dram_tensor("band", [128, 136], f32, kind="ExternalOutput")

    AO = mybir.AluOpType

    with tile.TileContext(nc) as tc:
        with tc.tile_pool(name="cpool", bufs=1) as cpool:
            # ---- M row-shard loads: A = rows 0..127, B = rows 128..255,
            # in column halves.  SEPARATE tiles per half: the tile framework
            # tracks dependencies at tile granularity, so a consumer of one
            # half must not share a tile with the other half's load.
            Ah = [cpool.tile([128, 2048], f32, name=f"A{h}", tag=f"A{h}") for h in range(2)]
            Bh = [cpool.tile([128, 2048], f32, name=f"B{h}", tag=f"B{h}") for h in range(2)]
            # ---- small inputs at the RING HEADS (before the big loads)
            # so their completion sems fire early — the gpsimd queue's
            # sems lag ~10us once the rings saturate.  kinds are converted
            # to f32 host-side (0..3, lossless) so no cast-DMA is needed.
            pt = cpool.tile([128, 4], f32)
            kt = cpool.tile([128, 4], f32)
            nc.sync.dma_start(out=pt[:], in_=params_s.ap()[:, :])
            nc.scalar.dma_start(out=kt[:], in_=kinds_s.ap()[:, :])

            nc.sync.dma_start(out=Ah[0][:], in_=m_rows.ap()[0:128, 0:2048])
            nc.scalar.dma_start(out=Bh[0][:], in_=m_rows.ap()[128:256, 0:2048])
            nc.sync.dma_start(out=Ah[1][:], in_=m_rows.ap()[0:128, 2048:4096])
            # B1 on sync too (3MB/1MB asymmetric loads): the scalar ring
            # empties at ~10.7us, and the gpsimd store queue co-flows at
            # 150-230 GB/s whenever only one ring is active — pulling the
            # store phase ~3us earlier
            nc.sync.dma_start(out=Bh[1][:], in_=m_rows.ap()[128:256, 2048:4096])


            # ---- band tile: identity block + z/y diagonal values
            bt = cpool.tile([128, 136], f32, tag="bt")
            ident = bt[:, 0:128]
            nc.gpsimd.memset(ident, 0.0)
            nc.gpsimd.affine_select(
                out=ident,
                in_=ident,
                compare_op=AO.not_equal,
                fill=1.0,
                base=0,
                pattern=[[-1, 128]],
                channel_multiplier=1,
            )

            # ---- kcl stores first on the gpsimd stream: their triggers
            # block on the load ring sems (~14us) and must issue before
            # the band small-ops so the store queue starts draining the
            # moment the sems fire instead of ~3us later.
            for h in range(2):
                sl = slice(h * 2048, (h + 1) * 2048)
                nc.gpsimd.dma_start(out=kcl.ap()[0:128, sl], in_=Ah[h][:])
                nc.gpsimd.dma_start(out=kcl.ap()[128:256, sl], in_=Bh[h][:])

            # ---- negated shard halves, ALL on DVE: the ACT engine shares
            # its instruction stream with the scalar DMA queue, so compute
            # there would stall DMA triggers behind it.  DVE is otherwise
            # idle; 4 x ~1.2us serial negates finish long before the bus
            # drains.
            Anh = [cpool.tile([128, 2048], f32, name=f"An{h}", tag=f"An{h}") for h in range(2)]
            Bnh = [cpool.tile([128, 2048], f32, name=f"Bn{h}", tag=f"Bn{h}") for h in range(2)]
            with tc.high_priority():
                for h in range(2):
                    nc.vector.tensor_scalar(
                        Anh[h][:], Ah[h][:], -1.0, None, op0=AO.mult
                    )
                    nc.vector.tensor_scalar(
                        Bnh[h][:], Bh[h][:], -1.0, None, op0=AO.mult
                    )

            # ---- z/y diagonal values (layout e_local = c*128 + p)
            rm = cpool.tile([128, 4], f32)
            im = cpool.tile([128, 4], f32)
            vm = cpool.tile([128, 4], f32)
            sm = cpool.tile([128, 4], f32)
            onm = cpool.tile([128, 4], f32)
            offm = cpool.tile([128, 4], f32)
            t0 = cpool.tile([128, 4], f32)
            t1 = cpool.tile([128, 4], f32)

            # all on the gpsimd (Pool) engine: keeps DVE free for the
            # negates (the scheduler would otherwise order these first on
            # DVE and stall the negates behind the slow pt/kt sems)
            nc.gpsimd.tensor_scalar(rm[:], kt[:], 0.0, None, op0=AO.is_equal)
            nc.gpsimd.tensor_scalar(im[:], kt[:], 1.0, None, op0=AO.is_equal)
            nc.gpsimd.tensor_scalar(vm[:], kt[:], 2.0, None, op0=AO.is_equal)
            nc.gpsimd.tensor_scalar(sm[:], kt[:], 3.0, None, op0=AO.is_equal)
            nc.gpsimd.tensor_scalar(onm[:], pt[:], 0.0, None, op0=AO.is_gt)
            nc.gpsimd.tensor_scalar(offm[:], pt[:], 0.0, None, op0=AO.is_le)
            # z = vc + sw*off - r*params
            nc.gpsimd.tensor_tensor(t0[:], sm[:], offm[:], op=AO.mult)
            nc.gpsimd.tensor_tensor(t0[:], vm[:], t0[:], op=AO.add)
            nc.gpsimd.tensor_tensor(t1[:], rm[:], pt[:], op=AO.mult)
            nc.gpsimd.tensor_tensor(bt[:, 128:132], t0[:], t1[:], op=AO.subtract)
            # y = r + ivs + sw*on
            nc.gpsimd.tensor_tensor(t0[:], sm[:], onm[:], op=AO.mult)
            nc.gpsimd.tensor_tensor(t0[:], im[:], t0[:], op=AO.add)
            nc.gpsimd.tensor_tensor(bt[:, 132:136], rm[:], t0[:], op=AO.add)
            # band on the sync ring, transferring mid-load-stream (its
            # trigger is sync's last item; the data is ready ~12.5us while
            # the loads still stream, so no idle and no small-descriptor
            # tail crawl)
            nc.sync.dma_start(out=band.ap()[:, :], in_=bt[:])

            # ---- ALL bulk stores on the gpsimd queue, in dependency-
            # readiness order (kcl waits load sems ~14us, mneg waits the
            # DVE negates ~15-18us).  The rings carry only the loads and
            # finish ~17.4us; the gpsimd queue then drains 8MB solo at
            # full rate — no mixed read/write phase, no ring-tail crawl,
            # and ~10.5us of SWDGE descriptor generation stays ahead of
            # the transfers.
            for h in range(2):
                sl = slice(h * 2048, (h + 1) * 2048)
                nc.gpsimd.dma_start(out=mneg.ap()[0:128, sl], in_=Anh[h][:])
                nc.gpsimd.dma_start(out=mneg.ap()[128:256, sl], in_=Bnh[h][:])


    nc.compile()
    return nc


def _get_nc(opts=None):
    key = ("nc", tuple(sorted((opts or {}).items())))
    if key not in _CACHE:
        _CACHE[key] = _build(opts)
    return _CACHE[key]


def _in_maps(M, params, kinds):
    maps = []
    for d in range(D):
        maps.append(
            {
                "m_rows": np.ascontiguousarray(M[d * NR : (d + 1) * NR, :]),
                "params_s": np.ascontiguousarray(
                    params[d * EC : (d + 1) * EC].reshape(4, 128).T
                ),
                "kinds_s": np.ascontiguousarray(
                    kinds[d * EC : (d + 1) * EC].reshape(4, 128).T.astype(np.float32)
                ),
            }
        )
    return maps


def kernel(M, params, kinds, _trace=False, _trace_kwargs=None, _opts=None):
    from concourse.bass_utils import run_bass_kernel_spmd

    M = np.ascontiguousarray(np.asarray(M, dtype=np.float32))
    params = np.ascontiguousarray(np.asarray(params, dtype=np.float32))
    kinds = np.ascontiguousarray(np.asarray(kinds, dtype=np.int32))
    assert M.shape == (N, E) and params.shape == (E,) and kinds.shape == (E,)

    nc = _get_nc(_opts)
    in_maps = _in_maps(M, params, kinds)
    # Warmup execution (once per process): the first run after model load
    # can race — the DMA ring semaphores carry residual values from the
    # load-time DMAs, so cross-queue waits can pass before data lands.
    # The first run's teardown resets every semaphore; subsequent runs
    # are reliable, so only the first call needs the warmup.
    if not _CACHE.get("warmed"):
        import os as _os

        _prev = _os.environ.get("BASS_NEVER_TRACE")
        _os.environ["BASS_NEVER_TRACE"] = "1"
        try:
            run_bass_kernel_spmd(nc, in_maps, core_ids=list(range(D)), trace=False)
        finally:
            if _prev is None:
                _os.environ.pop("BASS_NEVER_TRACE", None)
            else:
                _os.environ["BASS_NEVER_TRACE"] = _prev
        _CACHE["warmed"] = True
    res = run_bass_kernel_spmd(
        nc,
        in_maps,
        core_ids=list(range(D)),
        trace=_trace,
        **(_trace_kwargs or {}),
    )
    out = np.zeros((N + 2 * E, W), np.float32)
    ar = np.arange(EC)
    for d in range(D):
        r = res.results[d]
        out[d * NR : (d + 1) * NR, 0:E] = r["kcl"]
        # -M^T column block: transpose PLACEMENT of device-produced -M values
        out[N : N + E, 2 * E + d * NR : 2 * E + (d + 1) * NR] = r["mneg"].T
        eye = r["band"][:, 0:128]
        zvals = r["band"][:, 128:132].T.reshape(-1)
        yvals = r["band"][:, 132:136].T.reshape(-1)
        g0 = d * EC
        for c in range(4):
            b0 = g0 + c * 128
            out[N + b0 : N + b0 + 128, E + b0 : E + b0 + 128] = eye
        out[N + E + g0 + ar, g0 + ar] = zvals
        out[N + E + g0 + ar, E + g0 + ar] = yvals
    if _trace:
        _CACHE["last_result"] = res
    return out
